# revision 40
# baseline (speedup 1.0000x reference)
"""Causal self-attention on 8 trn2 NeuronCores.

Sharding: core c -> (batch b = c//2, head-group hg = c%2 of 8 heads).
Each core computes, for its batch and its 8 heads:
  qT,kT = (x[b] @ Wqk_shard).T        (q pre-scaled by 1/sqrt(hd))
  V     = x[b] @ Wv_shard
  S^T   = kT_h.T @ qT_h  per head     (s on partitions, t on free dim)
  P^T   = exp(S^T) with causal mask   (no max-subtraction: logits are O(5))
  yT    = V_aug.T @ P^T               (V carries a ones column -> row 64 = softmax denom)
  out_partial = y_local @ Wout_rows   ([T, E] fp32 partial sum)
Host: out[b] = partial[2b] + partial[2b+1] + b_out.

All matmul inputs fp16, PSUM accumulation fp32. x is pre-transposed and
pre-cast on host so no on-chip transpose is needed.
"""

import numpy as np

B, T, E, H, HD = 4, 2048, 1024, 16, 64
HPC = 8            # heads per core
DL = HPC * HD      # 512 local y dims per core
NT = T // 512      # 4 t-chunks of 512
NS = T // 128      # 16 s-tiles of 128
NE = E // 128      # 8 e-tiles

_CACHE = {}


def _make_tc_class():
    """TileContext whose tail drain splits sem waits across single-wait NOPs.

    The walrus build in this container rejects instructions carrying more
    than a couple of sync waits ("Too many sync wait commands" on the Tile
    tail Drain), so emit one NOP per logical proc, each with one wait.
    """
    import concourse.tile as tile
    from concourse.vector_clock import ScopedClock, VectorClock

    class TC(tile.TileContext):
        def _drain_and_barrier(self, tick_clock, wait_clock):
            gc = tick_clock.global_clock
            n = len(gc)
            for i in range(n):
                if gc[i] > 0:
                    vc = VectorClock([0] * n)
                    vc.require_at_least(i, gc[i])
                    nop = self.nc.sync.nop(nofuse=True)
                    wait_clock.add_sem_waits(nop.ins, ScopedClock({None: vc}))
            self.nc.sync.drain()
            self.nc.all_engine_barrier()
            assert self.sems is not None
            popped = self.nc._tile_sem_poison_stack.pop()
            assert popped is self._sem_poison
            self.nc.clear_and_free_semaphores(
                list(self.sems.allocated().values())
            )
            self.nc.all_engine_barrier()

    return TC


def _split_excess_waits(nc, max_waits=2):
    """Walrus in this container caps sem waits per instruction; hoist any
    excess waits onto fresh same-engine NOPs inserted just before."""
    import concourse.mybir as mybir

    n = 0
    for f in nc.m.functions:
        for bb in f.blocks:
            insts = bb.instructions
            out = []
            for inst in insts:
                si = inst.sync_info
                if si is not None and len(si.on_wait) > max_waits:
                    w = list(si.on_wait)
                    excess, keep = w[:-max_waits], w[-max_waits:]
                    for k in range(0, len(excess), max_waits):
                        nop = mybir.InstNoOp(
                            name=f"I-splitw-{n}", ins=[], outs=[]
                        )
                        n += 1
                        nop.engine = inst.engine
                        nop.sync_info = mybir.SyncInfo(
                            on_wait=excess[k:k + max_waits], on_update=[]
                        )
                        out.append(nop)
                    inst.sync_info = mybir.SyncInfo(
                        on_wait=keep, on_update=si.on_update
                    )
                out.append(inst)
            if n:
                bb.instructions = out
    return nc


def _build():
    import concourse.bass as bass
    import concourse.mybir as mybir

    dt = mybir.dt
    f16, f32 = dt.float16, dt.float32
    AF = mybir.ActivationFunctionType

    nc = bass.Bass()
    xt = nc.declare_dram_parameter("xt", [E, T], f16, isOutput=False)
    wqk = nc.declare_dram_parameter("wqk", [E, 1024], f16, isOutput=False)
    bqk = nc.declare_dram_parameter("bqk", [128, 8], f32, isOutput=False)
    wv = nc.declare_dram_parameter("wv", [E, 512], f16, isOutput=False)
    bv = nc.declare_dram_parameter("bv", [1, 512], f16, isOutput=False)
    wo = nc.declare_dram_parameter("wo", [DL, E], f16, isOutput=False)
    # maskadd[i, j] = 0 if j >= i else -30000 (additive causal mask)
    mask = nc.declare_dram_parameter("mask", [128, 128], f16, isOutput=False)
    ident = nc.declare_dram_parameter("ident", [128, 128], f16, isOutput=False)
    ones1 = nc.declare_dram_parameter("ones1", [1, 128], f16, isOutput=False)
    out = nc.declare_dram_parameter("out", [T, E], f32, isOutput=True)

    with _make_tc_class()(nc) as tc:
        with (
            tc.tile_pool(name="const", bufs=1) as constp,
            tc.tile_pool(name="xtp", bufs=1) as xtp,
            tc.tile_pool(name="wp", bufs=1) as wp,
            tc.tile_pool(name="qkv", bufs=1) as qkvp,
            tc.tile_pool(name="pt", bufs=6) as ptp,
            tc.tile_pool(name="rec", bufs=2) as recp,
            tc.tile_pool(name="stg", bufs=2) as stgp,
            tc.tile_pool(name="outp", bufs=3) as outp,
            tc.tile_pool(name="psA", bufs=2, space="PSUM") as psA,
            tc.tile_pool(name="psS", bufs=4, space="PSUM") as psS,
            tc.tile_pool(name="psY", bufs=2, space="PSUM") as psY,
        ):
            # ---- constants / weights ----
            bqk_sb = constp.tile([128, 8], f32, tag="bqk")
            nc.sync.dma_start(bqk_sb[:], bqk[:])
            bv_sb = constp.tile([1, 512], f16, tag="bv")
            nc.sync.dma_start(bv_sb[:], bv[:])
            mask_sb = constp.tile([128, 128], f16, tag="mask")
            nc.sync.dma_start(mask_sb[:], mask[:])
            ones_sb = constp.tile([1, 128], f16, tag="ones1")
            nc.sync.dma_start(ones_sb[:], ones1[:])
            ident_sb = constp.tile([128, 128], f16, tag="ident")
            nc.sync.dma_start(ident_sb[:], ident[:])

            xt_sb = []
            for i in range(NE):
                t_ = xtp.tile([128, T], f16, tag=f"xt{i}")
                nc.sync.dma_start(t_[:], xt[i * 128:(i + 1) * 128, :])
                xt_sb.append(t_)
            wqk_sb = []
            for i in range(NE):
                t_ = wp.tile([128, 1024], f16, tag=f"wqk{i}")
                nc.sync.dma_start(t_[:], wqk[i * 128:(i + 1) * 128, :])
                wqk_sb.append(t_)
            wv_sb = []
            for i in range(NE):
                t_ = wp.tile([128, 512], f16, tag=f"wv{i}")
                nc.sync.dma_start(t_[:], wv[i * 128:(i + 1) * 128, :])
                wv_sb.append(t_)
            wo_sb = []
            for i in range(4):
                t_ = wp.tile([128, 1024], f16, tag=f"wo{i}")
                nc.sync.dma_start(t_[:], wo[i * 128:(i + 1) * 128, :])
                wo_sb.append(t_)

            qt_sb = [qkvp.tile([128, T], f16, tag=f"qt{i}", name=f"qt{i}") for i in range(4)]
            kt_sb = [qkvp.tile([128, T], f16, tag=f"kt{i}", name=f"kt{i}") for i in range(4)]
            yt_sb = [qkvp.tile([128, T], f16, tag=f"yt{i}", name=f"yt{i}") for i in range(4)]
            va_sb = [qkvp.tile([128, 8, 65], f16, tag=f"va{i}", name=f"va{i}") for i in range(NS)]

            from concourse.tile import add_dep_helper

            pend = []

            def _flush_tail(item, anchor):
                """Emit the deferred softmax-divide tail. `anchor` is a PE
                instruction the broadcast matmul is ordered after, giving the
                DVE reciprocal time to finish without stalling the PE."""
                rec, yr, hp_, qrow_, tcx_ = item
                bps = psA.tile([64, 512], f32, tag="psA", name="bps")
                mm = nc.tensor.matmul(
                    bps[:], ones_sb[:, 0:64], rec[:], start=True, stop=True
                )
                if anchor is not None:
                    add_dep_helper(
                        mm.ins, anchor.ins, reason="defer tail past anchor"
                    )
                bcs = stgp.tile([64, 512], f32, tag="bcs", bufs=3, name="bcs")
                nc.vector.tensor_copy(bcs[:], bps[:])
                nc.vector.tensor_mul(
                    yt_sb[hp_][qrow_, tcx_ * 512:(tcx_ + 1) * 512],
                    yr[0:64, :],
                    bcs[:],
                )

            # t-chunk-outer structure: projections for chunk tcx, then
            # attention for all heads at tcx (keys/values <= tcx are ready),
            # then the output projection for tcx's t-tiles.  The scheduler
            # can interleave across sections to keep the PE stream dense.
            def _oproj(tcx):
                for tt in range(4 * tcx, 4 * tcx + 4):
                    for cc in range(2):
                        ps = psA.tile([128, 512], f32, tag="psA", name="ops")
                        for hp in range(4):
                            nc.tensor.matmul(
                                ps[:],
                                yt_sb[hp][:, tt * 128:(tt + 1) * 128],
                                wo_sb[hp][:, cc * 512:(cc + 1) * 512],
                                start=(hp == 0),
                                stop=(hp == 3),
                            )
                        osb = outp.tile([128, 512], f32, tag="osb", name="osb")
                        nc.vector.tensor_copy(osb[:], ps[:])
                        nc.sync.dma_start(
                            out[tt * 128:(tt + 1) * 128,
                                cc * 512:(cc + 1) * 512],
                            osb[:],
                        )

            for tcx in range(NT):
                # -- qT/kT projection for this t-chunk --
                anchor_mid = None
                for jt in range(8):
                    dest = qt_sb[jt] if jt < 4 else kt_sb[jt - 4]
                    ps = psA.tile([128, 512], f32, tag="psA")
                    for et in range(NE):
                        mm = nc.tensor.matmul(
                            ps[:],
                            wqk_sb[et][:, jt * 128:(jt + 1) * 128],
                            xt_sb[et][:, tcx * 512:(tcx + 1) * 512],
                            start=(et == 0),
                            stop=(et == NE - 1),
                        )
                    if jt == 3:
                        anchor_mid = mm
                    nc.vector.tensor_scalar_add(
                        dest[:, tcx * 512:(tcx + 1) * 512], ps[:],
                        bqk_sb[:, jt:jt + 1],
                    )

                # previous chunk's last head tail + output projection, placed
                # here so its reciprocal hides under this chunk's projections
                if pend:
                    _flush_tail(pend.pop(), anchor_mid)
                if tcx > 0:
                    _oproj(tcx - 1)

                # -- V projection for this chunk's 4 s-tiles --
                for st in range(4 * tcx, 4 * tcx + 4):
                    ps = psA.tile([128, 512], f32, tag="psA")
                    for et in range(NE):
                        nc.tensor.matmul(
                            ps[:],
                            xt_sb[et][:, st * 128:(st + 1) * 128],
                            wv_sb[et][:],
                            start=(et == 0),
                            stop=False,
                        )
                    # bias row: V += 1 * bv
                    nc.tensor.matmul(
                        ps[:], ones_sb[:], bv_sb[:], start=False, stop=True,
                    )
                    va = va_sb[st]
                    nc.vector.tensor_copy(
                        va[:, :, 0:64],
                        ps[:].rearrange("p (h c) -> p h c", c=64),
                    )
                    nc.vector.memset(va[:, :, 64:65], 1.0)

                # -- attention for all heads at this t-chunk --
                # The per-head tail (reciprocal -> broadcast-matmul -> divide)
                # is deferred into the middle of the NEXT head's matmul stream
                # so the in-order PE never stalls on the 3.3us DVE reciprocal.
                nst = 4 * (tcx + 1)
                for h in range(HPC):
                    hp, ho = divmod(h, 2)
                    qrow = slice(ho * 64, (ho + 1) * 64)
                    yps = psY.tile([65, 512], f32, tag="psY")
                    for g in range(0, nst, 4):
                        gn = min(4, nst - g)
                        sgrp = []
                        for st in range(g, g + gn):
                            diag = st * 128 >= tcx * 512
                            sps = psS.tile([128, 512], f32, tag="psS")
                            nc.tensor.matmul(
                                sps[:],
                                kt_sb[hp][qrow, st * 128:(st + 1) * 128],
                                qt_sb[hp][qrow, tcx * 512:(tcx + 1) * 512],
                                start=True,
                                stop=not diag,
                            )
                            lo = max(0, st * 128 - tcx * 512)
                            if diag:
                                # additive causal mask on the diagonal window
                                nc.tensor.matmul(
                                    sps[:, lo:lo + 128],
                                    ident_sb[:],
                                    mask_sb[:],
                                    start=False,
                                    stop=True,
                                )
                            pt = ptp.tile([128, 512], f16, tag="pt")
                            nc.scalar.activation(
                                pt[:, lo:512], sps[:, lo:512], AF.Exp
                            )
                            sgrp.append((st, lo, pt))
                        av_last = None
                        for st, lo, pt in sgrp:
                            av_last = nc.tensor.matmul(
                                yps[:, lo:512],
                                va_sb[st][:, h, :],
                                pt[:, lo:512],
                                start=(st == 0),
                                stop=(st == nst - 1),
                            )
                        if pend and (g == 4 or (g == 0 and nst == 4)):
                            _flush_tail(pend.pop(), av_last)
                    # evacuate the accumulator to SBUF on ACT so the PSUM bank
                    # frees immediately; 1/den = exp(-ln(den)) stays on ACT
                    yr = stgp.tile([65, 512], f32, tag="yr", bufs=3, name="yr")
                    nc.scalar.activation(yr[:], yps[:], AF.Copy)
                    lnd = recp.tile([1, 512], f32, tag="lnd", bufs=3)
                    nc.scalar.activation(lnd[:], yr[64:65, :], AF.Ln)
                    rec = recp.tile([1, 512], f16, tag="rec", bufs=3)
                    nc.scalar.activation(rec[:], lnd[:], AF.Exp, scale=-1.0)
                    pend.append((rec, yr, hp, qrow, tcx))

            # final chunk: last head's tail (short PE stall) + last oproj
            if pend:
                _flush_tail(pend.pop(), None)
            _oproj(NT - 1)
    return _split_excess_waits(nc, max_waits=1)


def _prep_in_maps(x, W_qkv, b_qkv, W_out):
    f16 = np.float16
    x = np.asarray(x, np.float32)
    W_qkv = np.asarray(W_qkv, np.float32)
    b_qkv = np.asarray(b_qkv, np.float32)
    W_out = np.asarray(W_out, np.float32)

    mask = np.where(
        np.triu(np.ones((128, 128), dtype=bool)), 0.0, -30000.0
    ).astype(f16)
    ident = np.eye(128, dtype=f16)
    ones1 = np.ones((1, 128), dtype=f16)
    in_maps = []
    for c in range(8):
        b, hg = divmod(c, 2)
        qs = slice(hg * 512, (hg + 1) * 512)
        ks = slice(E + hg * 512, E + (hg + 1) * 512)
        vs = slice(2 * E + hg * 512, 2 * E + (hg + 1) * 512)
        wqk_c = np.concatenate(
            [W_qkv[:, qs] * 0.125, W_qkv[:, ks]], axis=1
        ).astype(f16)
        bqk_c = np.concatenate(
            [b_qkv[qs] * 0.125, b_qkv[ks]]
        ).astype(np.float32).reshape(8, 128).T.copy()
        in_maps.append({
            "xt": np.ascontiguousarray(x[b].T).astype(f16),
            "ident": ident,
            "wqk": wqk_c,
            "bqk": bqk_c,
            "wv": W_qkv[:, vs].astype(f16),
            "bv": b_qkv[vs].astype(f16).reshape(1, 512),
            "wo": W_out[hg * 512:(hg + 1) * 512, :].astype(f16),
            "mask": mask,
            "ones1": ones1,
        })
    return in_maps


def run(x, W_qkv, b_qkv, W_out, b_out, trace=False, **trace_kwargs):
    from concourse.bass_utils import run_bass_kernel_spmd

    if "nc" not in _CACHE:
        _CACHE["nc"] = _build()
    nc = _CACHE["nc"]
    in_maps = _prep_in_maps(x, W_qkv, b_qkv, W_out)
    res = run_bass_kernel_spmd(
        nc, in_maps, list(range(8)), trace=trace, **trace_kwargs
    )
    parts = [r["out"] for r in res.results]
    b_out = np.asarray(b_out, np.float32)
    y = np.stack([parts[2 * b] + parts[2 * b + 1] for b in range(B)]) + b_out
    return y.astype(np.float32), res


def kernel(x, W_qkv, b_qkv, W_out, b_out):
    y, _ = run(x, W_qkv, b_qkv, W_out, b_out, trace=False)
    return y


# revision 42
# speedup vs baseline: 1.0226x; 1.0226x over previous
"""Causal self-attention on 8 trn2 NeuronCores.

Sharding: core c -> (batch b = c//2, head-group hg = c%2 of 8 heads).
Each core computes, for its batch and its 8 heads:
  qT,kT = (x[b] @ Wqk_shard).T        (q pre-scaled by 1/sqrt(hd))
  V     = x[b] @ Wv_shard
  S^T   = kT_h.T @ qT_h  per head     (s on partitions, t on free dim)
  P^T   = exp(S^T) with causal mask   (no max-subtraction: logits are O(5))
  yT    = V_aug.T @ P^T               (V carries a ones column -> row 64 = softmax denom)
  out_partial = y_local @ Wout_rows   ([T, E] fp32 partial sum)
Host: out[b] = partial[2b] + partial[2b+1] + b_out.

All matmul inputs fp16, PSUM accumulation fp32. x is pre-transposed and
pre-cast on host so no on-chip transpose is needed.
"""

import numpy as np

B, T, E, H, HD = 4, 2048, 1024, 16, 64
HPC = 8            # heads per core
DL = HPC * HD      # 512 local y dims per core
NT = T // 512      # 4 t-chunks of 512
NS = T // 128      # 16 s-tiles of 128
NE = E // 128      # 8 e-tiles

_CACHE = {}


def _make_tc_class():
    """TileContext whose tail drain splits sem waits across single-wait NOPs.

    The walrus build in this container rejects instructions carrying more
    than a couple of sync waits ("Too many sync wait commands" on the Tile
    tail Drain), so emit one NOP per logical proc, each with one wait.
    """
    import concourse.tile as tile
    from concourse.vector_clock import ScopedClock, VectorClock

    class TC(tile.TileContext):
        def _drain_and_barrier(self, tick_clock, wait_clock):
            gc = tick_clock.global_clock
            n = len(gc)
            for i in range(n):
                if gc[i] > 0:
                    vc = VectorClock([0] * n)
                    vc.require_at_least(i, gc[i])
                    nop = self.nc.sync.nop(nofuse=True)
                    wait_clock.add_sem_waits(nop.ins, ScopedClock({None: vc}))
            self.nc.sync.drain()
            self.nc.all_engine_barrier()
            assert self.sems is not None
            popped = self.nc._tile_sem_poison_stack.pop()
            assert popped is self._sem_poison
            self.nc.clear_and_free_semaphores(
                list(self.sems.allocated().values())
            )
            self.nc.all_engine_barrier()

    return TC


def _split_excess_waits(nc, max_waits=2):
    """Walrus in this container caps sem waits per instruction; hoist any
    excess waits onto fresh same-engine NOPs inserted just before."""
    import concourse.mybir as mybir

    n = 0
    for f in nc.m.functions:
        for bb in f.blocks:
            insts = bb.instructions
            out = []
            for inst in insts:
                si = inst.sync_info
                if si is not None and len(si.on_wait) > max_waits:
                    w = list(si.on_wait)
                    excess, keep = w[:-max_waits], w[-max_waits:]
                    for k in range(0, len(excess), max_waits):
                        nop = mybir.InstNoOp(
                            name=f"I-splitw-{n}", ins=[], outs=[]
                        )
                        n += 1
                        nop.engine = inst.engine
                        nop.sync_info = mybir.SyncInfo(
                            on_wait=excess[k:k + max_waits], on_update=[]
                        )
                        out.append(nop)
                    inst.sync_info = mybir.SyncInfo(
                        on_wait=keep, on_update=si.on_update
                    )
                out.append(inst)
            if n:
                bb.instructions = out
    return nc


def _build():
    import concourse.bass as bass
    import concourse.mybir as mybir

    dt = mybir.dt
    f16, f32 = dt.float16, dt.float32
    AF = mybir.ActivationFunctionType

    nc = bass.Bass()
    xt = nc.declare_dram_parameter("xt", [E, T], f16, isOutput=False)
    wqk = nc.declare_dram_parameter("wqk", [E, 1024], f16, isOutput=False)
    bqk = nc.declare_dram_parameter("bqk", [128, 8], f32, isOutput=False)
    wv = nc.declare_dram_parameter("wv", [E, 512], f16, isOutput=False)
    bv = nc.declare_dram_parameter("bv", [1, 512], f16, isOutput=False)
    wo = nc.declare_dram_parameter("wo", [DL, E], f16, isOutput=False)
    # maskadd[i, j] = 0 if j >= i else -30000 (additive causal mask)
    mask = nc.declare_dram_parameter("mask", [128, 128], f16, isOutput=False)
    ident = nc.declare_dram_parameter("ident", [128, 128], f16, isOutput=False)
    ones1 = nc.declare_dram_parameter("ones1", [1, 128], f16, isOutput=False)
    out = nc.declare_dram_parameter("out", [T, E], f32, isOutput=True)

    with _make_tc_class()(nc) as tc:
        with (
            tc.tile_pool(name="const", bufs=1) as constp,
            tc.tile_pool(name="xtp", bufs=1) as xtp,
            tc.tile_pool(name="wp", bufs=1) as wp,
            tc.tile_pool(name="qkv", bufs=1) as qkvp,
            tc.tile_pool(name="pt", bufs=6) as ptp,
            tc.tile_pool(name="rec", bufs=2) as recp,
            tc.tile_pool(name="stg", bufs=2) as stgp,
            tc.tile_pool(name="outp", bufs=3) as outp,
            tc.tile_pool(name="psA", bufs=2, space="PSUM") as psA,
            tc.tile_pool(name="psS", bufs=4, space="PSUM") as psS,
            tc.tile_pool(name="psY", bufs=2, space="PSUM") as psY,
        ):
            # ---- constants / weights ----
            bqk_sb = constp.tile([128, 8], f32, tag="bqk")
            nc.sync.dma_start(bqk_sb[:], bqk[:])
            bv_sb = constp.tile([1, 512], f16, tag="bv")
            nc.sync.dma_start(bv_sb[:], bv[:])
            mask_sb = constp.tile([128, 128], f16, tag="mask")
            nc.sync.dma_start(mask_sb[:], mask[:])
            ones_sb = constp.tile([1, 128], f16, tag="ones1")
            nc.sync.dma_start(ones_sb[:], ones1[:])
            ident_sb = constp.tile([128, 128], f16, tag="ident")
            nc.sync.dma_start(ident_sb[:], ident[:])

            xt_sb = []
            for i in range(NE):
                t_ = xtp.tile([128, T], f16, tag=f"xt{i}")
                nc.sync.dma_start(t_[:], xt[i * 128:(i + 1) * 128, :])
                xt_sb.append(t_)
            wqk_sb = []
            for i in range(NE):
                t_ = wp.tile([128, 1024], f16, tag=f"wqk{i}")
                nc.sync.dma_start(t_[:], wqk[i * 128:(i + 1) * 128, :])
                wqk_sb.append(t_)
            wv_sb = []
            for i in range(NE):
                t_ = wp.tile([128, 512], f16, tag=f"wv{i}")
                nc.sync.dma_start(t_[:], wv[i * 128:(i + 1) * 128, :])
                wv_sb.append(t_)
            wo_sb = []
            for i in range(4):
                t_ = wp.tile([128, 1024], f16, tag=f"wo{i}")
                nc.sync.dma_start(t_[:], wo[i * 128:(i + 1) * 128, :])
                wo_sb.append(t_)

            qt_sb = [qkvp.tile([128, T], f16, tag=f"qt{i}", name=f"qt{i}") for i in range(4)]
            kt_sb = [qkvp.tile([128, T], f16, tag=f"kt{i}", name=f"kt{i}") for i in range(4)]
            yt_sb = [qkvp.tile([128, T], f16, tag=f"yt{i}", name=f"yt{i}") for i in range(4)]
            va_sb = [qkvp.tile([128, 8, 65], f16, tag=f"va{i}", name=f"va{i}") for i in range(NS)]

            from concourse.tile import add_dep_helper

            pend = []

            def _flush_tail(item, anchor):
                """Emit the deferred softmax-divide tail. `anchor` is a PE
                instruction the broadcast matmul is ordered after, giving the
                DVE reciprocal time to finish without stalling the PE."""
                rec, yr, hp_, qrow_, tcx_ = item
                bps = psA.tile([64, 512], f32, tag="psA", name="bps")
                mm = nc.tensor.matmul(
                    bps[:], ones_sb[:, 0:64], rec[:], start=True, stop=True
                )
                if anchor is not None:
                    add_dep_helper(
                        mm.ins, anchor.ins, reason="defer tail past anchor"
                    )
                bcs = stgp.tile([64, 512], f32, tag="bcs", bufs=3, name="bcs")
                nc.vector.tensor_copy(bcs[:], bps[:])
                nc.vector.tensor_mul(
                    yt_sb[hp_][qrow_, tcx_ * 512:(tcx_ + 1) * 512],
                    yr[0:64, :],
                    bcs[:],
                )

            # t-chunk-outer structure: projections for chunk tcx, then
            # attention for all heads at tcx (keys/values <= tcx are ready),
            # then the output projection for tcx's t-tiles.  The scheduler
            # can interleave across sections to keep the PE stream dense.
            def _oproj(tcx):
                for tt in range(4 * tcx, 4 * tcx + 4):
                    for cc in range(2):
                        ps = psA.tile([128, 512], f32, tag="psA", name="ops")
                        for hp in range(4):
                            nc.tensor.matmul(
                                ps[:],
                                yt_sb[hp][:, tt * 128:(tt + 1) * 128],
                                wo_sb[hp][:, cc * 512:(cc + 1) * 512],
                                start=(hp == 0),
                                stop=(hp == 3),
                            )
                        osb = outp.tile([128, 512], f32, tag="osb", name="osb")
                        nc.vector.tensor_copy(osb[:], ps[:])
                        nc.sync.dma_start(
                            out[tt * 128:(tt + 1) * 128,
                                cc * 512:(cc + 1) * 512],
                            osb[:],
                        )

            for tcx in range(NT):
                # -- qT/kT projection for this t-chunk --
                anchor_mid = None
                for jt in range(8):
                    dest = qt_sb[jt] if jt < 4 else kt_sb[jt - 4]
                    ps = psA.tile([128, 512], f32, tag="psA")
                    for et in range(NE):
                        mm = nc.tensor.matmul(
                            ps[:],
                            wqk_sb[et][:, jt * 128:(jt + 1) * 128],
                            xt_sb[et][:, tcx * 512:(tcx + 1) * 512],
                            start=(et == 0),
                            stop=(et == NE - 1),
                        )
                    if jt == 3:
                        anchor_mid = mm
                    nc.vector.tensor_scalar_add(
                        dest[:, tcx * 512:(tcx + 1) * 512], ps[:],
                        bqk_sb[:, jt:jt + 1],
                    )

                # previous chunk's last head tail + output projection, placed
                # here so its reciprocal hides under this chunk's projections
                while pend:
                    _flush_tail(pend.pop(0), anchor_mid)
                if tcx > 0:
                    _oproj(tcx - 1)

                # -- V projection for this chunk's 4 s-tiles --
                for st in range(4 * tcx, 4 * tcx + 4):
                    ps = psA.tile([128, 512], f32, tag="psA")
                    for et in range(NE):
                        nc.tensor.matmul(
                            ps[:],
                            xt_sb[et][:, st * 128:(st + 1) * 128],
                            wv_sb[et][:],
                            start=(et == 0),
                            stop=False,
                        )
                    # bias row: V += 1 * bv
                    nc.tensor.matmul(
                        ps[:], ones_sb[:], bv_sb[:], start=False, stop=True,
                    )
                    va = va_sb[st]
                    nc.vector.tensor_copy(
                        va[:, :, 0:64],
                        ps[:].rearrange("p (h c) -> p h c", c=64),
                    )
                    nc.vector.memset(va[:, :, 64:65], 1.0)

                # -- attention, head pairs in lockstep --
                # The two heads of a pair live on partition halves 0-63/64-127
                # of qt/kt, so interleaving their S^T matmuls alternates PE
                # row-groups and LDWEIGHTS pulls ahead instead of serializing.
                # Each head's softmax-divide tail is deferred into a later
                # stream position so the PE never waits on the DVE reciprocal.
                nst = 4 * (tcx + 1)
                for hp in range(4):
                    ypses = [
                        psY.tile([65, 512], f32, tag="psY", name=f"yps{p}")
                        for p in range(2)
                    ]
                    for g in range(0, nst, 2):
                        gn = min(2, nst - g)
                        sgrp = []
                        for st in range(g, g + gn):
                            diag = st * 128 >= tcx * 512
                            lo = max(0, st * 128 - tcx * 512)
                            for p in range(2):
                                qrow = slice(p * 64, (p + 1) * 64)
                                sps = psS.tile([128, 512], f32, tag="psS")
                                nc.tensor.matmul(
                                    sps[:],
                                    kt_sb[hp][qrow, st * 128:(st + 1) * 128],
                                    qt_sb[hp][qrow,
                                              tcx * 512:(tcx + 1) * 512],
                                    start=True,
                                    stop=not diag,
                                )
                                if diag:
                                    nc.tensor.matmul(
                                        sps[:, lo:lo + 128],
                                        ident_sb[:],
                                        mask_sb[:],
                                        start=False,
                                        stop=True,
                                    )
                                pt = ptp.tile([128, 512], f16, tag="pt")
                                nc.scalar.activation(
                                    pt[:, lo:512], sps[:, lo:512], AF.Exp
                                )
                                sgrp.append((p, st, lo, pt))
                        av_last = None
                        for p, st, lo, pt in sgrp:
                            av_last = nc.tensor.matmul(
                                ypses[p][:, lo:512],
                                va_sb[st][:, 2 * hp + p, :],
                                pt[:, lo:512],
                                start=(st == 0),
                                stop=(st == nst - 1),
                            )
                        if pend and g in (2, 4):
                            _flush_tail(pend.pop(0), av_last)
                    for p in range(2):
                        yps = ypses[p]
                        qrow = slice(p * 64, (p + 1) * 64)
                        # evacuate accumulator on ACT (frees the PSUM bank);
                        # reciprocal + divide run from SBUF
                        yr = stgp.tile(
                            [65, 512], f32, tag="yr", bufs=4, name="yr"
                        )
                        nc.scalar.activation(yr[:], yps[:], AF.Copy)
                        rec = recp.tile([1, 512], f16, tag="rec", bufs=4)
                        with nc.allow_low_precision(reason="f16 smax recip"):
                            nc.vector.reciprocal(rec[:], yr[64:65, :])
                        pend.append((rec, yr, hp, qrow, tcx))

            # final chunk: remaining tails (short PE stall) + last oproj
            while pend:
                _flush_tail(pend.pop(0), None)
            _oproj(NT - 1)
    return _split_excess_waits(nc, max_waits=1)


def _prep_in_maps(x, W_qkv, b_qkv, W_out):
    f16 = np.float16
    x = np.asarray(x, np.float32)
    W_qkv = np.asarray(W_qkv, np.float32)
    b_qkv = np.asarray(b_qkv, np.float32)
    W_out = np.asarray(W_out, np.float32)

    mask = np.where(
        np.triu(np.ones((128, 128), dtype=bool)), 0.0, -30000.0
    ).astype(f16)
    ident = np.eye(128, dtype=f16)
    ones1 = np.ones((1, 128), dtype=f16)
    in_maps = []
    for c in range(8):
        b, hg = divmod(c, 2)
        qs = slice(hg * 512, (hg + 1) * 512)
        ks = slice(E + hg * 512, E + (hg + 1) * 512)
        vs = slice(2 * E + hg * 512, 2 * E + (hg + 1) * 512)
        wqk_c = np.concatenate(
            [W_qkv[:, qs] * 0.125, W_qkv[:, ks]], axis=1
        ).astype(f16)
        bqk_c = np.concatenate(
            [b_qkv[qs] * 0.125, b_qkv[ks]]
        ).astype(np.float32).reshape(8, 128).T.copy()
        in_maps.append({
            "xt": np.ascontiguousarray(x[b].T).astype(f16),
            "ident": ident,
            "wqk": wqk_c,
            "bqk": bqk_c,
            "wv": W_qkv[:, vs].astype(f16),
            "bv": b_qkv[vs].astype(f16).reshape(1, 512),
            "wo": W_out[hg * 512:(hg + 1) * 512, :].astype(f16),
            "mask": mask,
            "ones1": ones1,
        })
    return in_maps


def run(x, W_qkv, b_qkv, W_out, b_out, trace=False, **trace_kwargs):
    from concourse.bass_utils import run_bass_kernel_spmd

    if "nc" not in _CACHE:
        _CACHE["nc"] = _build()
    nc = _CACHE["nc"]
    in_maps = _prep_in_maps(x, W_qkv, b_qkv, W_out)
    res = run_bass_kernel_spmd(
        nc, in_maps, list(range(8)), trace=trace, **trace_kwargs
    )
    parts = [r["out"] for r in res.results]
    b_out = np.asarray(b_out, np.float32)
    y = np.stack([parts[2 * b] + parts[2 * b + 1] for b in range(B)]) + b_out
    return y.astype(np.float32), res


def kernel(x, W_qkv, b_qkv, W_out, b_out):
    y, _ = run(x, W_qkv, b_qkv, W_out, b_out, trace=False)
    return y


# revision 43
# speedup vs baseline: 1.2285x; 1.2014x over previous
"""Causal self-attention on 8 trn2 NeuronCores.

Sharding: core c -> (batch b = c//2, head-group hg = c%2 of 8 heads).
Each core computes, for its batch and its 8 heads:
  qT,kT = (x[b] @ Wqk_shard).T        (q pre-scaled by 1/sqrt(hd))
  V     = x[b] @ Wv_shard
  S^T   = kT_h.T @ qT_h  per head     (s on partitions, t on free dim)
  P^T   = exp(S^T) with causal mask   (no max-subtraction: logits are O(5))
  yT    = V_aug.T @ P^T               (V carries a ones column -> row 64 = softmax denom)
  out_partial = y_local @ Wout_rows   ([T, E] fp32 partial sum)
Host: out[b] = partial[2b] + partial[2b+1] + b_out.

All matmul inputs fp16, PSUM accumulation fp32. x is pre-transposed and
pre-cast on host so no on-chip transpose is needed.
"""

import numpy as np

B, T, E, H, HD = 4, 2048, 1024, 16, 64
HPC = 8            # heads per core
DL = HPC * HD      # 512 local y dims per core
NT = T // 512      # 4 t-chunks of 512
NS = T // 128      # 16 s-tiles of 128
NE = E // 128      # 8 e-tiles

_CACHE = {}


def _make_tc_class():
    """TileContext whose tail drain splits sem waits across single-wait NOPs.

    The walrus build in this container rejects instructions carrying more
    than a couple of sync waits ("Too many sync wait commands" on the Tile
    tail Drain), so emit one NOP per logical proc, each with one wait.
    """
    import concourse.tile as tile
    from concourse.vector_clock import ScopedClock, VectorClock

    class TC(tile.TileContext):
        def _drain_and_barrier(self, tick_clock, wait_clock):
            gc = tick_clock.global_clock
            n = len(gc)
            for i in range(n):
                if gc[i] > 0:
                    vc = VectorClock([0] * n)
                    vc.require_at_least(i, gc[i])
                    nop = self.nc.sync.nop(nofuse=True)
                    wait_clock.add_sem_waits(nop.ins, ScopedClock({None: vc}))
            self.nc.sync.drain()
            self.nc.all_engine_barrier()
            assert self.sems is not None
            popped = self.nc._tile_sem_poison_stack.pop()
            assert popped is self._sem_poison
            self.nc.clear_and_free_semaphores(
                list(self.sems.allocated().values())
            )
            self.nc.all_engine_barrier()

    return TC


def _split_excess_waits(nc, max_waits=2):
    """Walrus in this container caps sem waits per instruction; hoist any
    excess waits onto fresh same-engine NOPs inserted just before."""
    import concourse.mybir as mybir

    n = 0
    for f in nc.m.functions:
        for bb in f.blocks:
            insts = bb.instructions
            out = []
            for inst in insts:
                si = inst.sync_info
                if si is not None and len(si.on_wait) > max_waits:
                    w = list(si.on_wait)
                    excess, keep = w[:-max_waits], w[-max_waits:]
                    for k in range(0, len(excess), max_waits):
                        nop = mybir.InstNoOp(
                            name=f"I-splitw-{n}", ins=[], outs=[]
                        )
                        n += 1
                        nop.engine = inst.engine
                        nop.sync_info = mybir.SyncInfo(
                            on_wait=excess[k:k + max_waits], on_update=[]
                        )
                        out.append(nop)
                    inst.sync_info = mybir.SyncInfo(
                        on_wait=keep, on_update=si.on_update
                    )
                out.append(inst)
            if n:
                bb.instructions = out
    return nc


def _build():
    import concourse.bass as bass
    import concourse.mybir as mybir

    dt = mybir.dt
    f16, f32 = dt.float16, dt.float32
    AF = mybir.ActivationFunctionType

    nc = bass.Bass()
    xt = nc.declare_dram_parameter("xt", [E, T], f16, isOutput=False)
    wqk = nc.declare_dram_parameter("wqk", [E, 1024], f16, isOutput=False)
    bqk = nc.declare_dram_parameter("bqk", [128, 8], f32, isOutput=False)
    wv = nc.declare_dram_parameter("wv", [E, 512], f16, isOutput=False)
    bv = nc.declare_dram_parameter("bv", [1, 512], f16, isOutput=False)
    wo = nc.declare_dram_parameter("wo", [DL, E], f16, isOutput=False)
    # maskadd[i, j] = 0 if j >= i else -30000 (additive causal mask)
    mask = nc.declare_dram_parameter("mask", [128, 128], f16, isOutput=False)
    ident = nc.declare_dram_parameter("ident", [128, 128], f16, isOutput=False)
    ones1 = nc.declare_dram_parameter("ones1", [1, 128], f16, isOutput=False)
    out = nc.declare_dram_parameter("out", [T, E], f32, isOutput=True)

    with _make_tc_class()(nc) as tc:
        with (
            tc.tile_pool(name="const", bufs=1) as constp,
            tc.tile_pool(name="xtp", bufs=1) as xtp,
            tc.tile_pool(name="wp", bufs=1) as wp,
            tc.tile_pool(name="qkv", bufs=1) as qkvp,
            tc.tile_pool(name="pt", bufs=6) as ptp,
            tc.tile_pool(name="rec", bufs=2) as recp,
            tc.tile_pool(name="stg", bufs=2) as stgp,
            tc.tile_pool(name="outp", bufs=3) as outp,
            tc.tile_pool(name="psA", bufs=2, space="PSUM") as psA,
            tc.tile_pool(name="psS", bufs=4, space="PSUM") as psS,
            tc.tile_pool(name="psY", bufs=2, space="PSUM") as psY,
        ):
            # ---- constants / weights ----
            bqk_sb = constp.tile([128, 8], f32, tag="bqk")
            nc.sync.dma_start(bqk_sb[:], bqk[:])
            bv_sb = constp.tile([1, 512], f16, tag="bv")
            nc.sync.dma_start(bv_sb[:], bv[:])
            mask_sb = constp.tile([128, 128], f16, tag="mask")
            nc.sync.dma_start(mask_sb[:], mask[:])
            ones_sb = constp.tile([1, 128], f16, tag="ones1")
            nc.sync.dma_start(ones_sb[:], ones1[:])
            ident_sb = constp.tile([128, 128], f16, tag="ident")
            nc.sync.dma_start(ident_sb[:], ident[:])

            xt_sb = []
            for i in range(NE):
                t_ = xtp.tile([128, T], f16, tag=f"xt{i}")
                nc.sync.dma_start(t_[:], xt[i * 128:(i + 1) * 128, :])
                xt_sb.append(t_)
            wqk_sb = []
            for i in range(NE):
                t_ = wp.tile([128, 1024], f16, tag=f"wqk{i}")
                nc.sync.dma_start(t_[:], wqk[i * 128:(i + 1) * 128, :])
                wqk_sb.append(t_)
            wv_sb = []
            for i in range(NE):
                t_ = wp.tile([128, 512], f16, tag=f"wv{i}")
                nc.sync.dma_start(t_[:], wv[i * 128:(i + 1) * 128, :])
                wv_sb.append(t_)
            wo_sb = []
            for i in range(4):
                t_ = wp.tile([128, 1024], f16, tag=f"wo{i}")
                nc.sync.dma_start(t_[:], wo[i * 128:(i + 1) * 128, :])
                wo_sb.append(t_)

            qt_sb = [qkvp.tile([128, T], f16, tag=f"qt{i}", name=f"qt{i}") for i in range(4)]
            kt_sb = [qkvp.tile([128, T], f16, tag=f"kt{i}", name=f"kt{i}") for i in range(4)]
            yt_sb = [qkvp.tile([128, T], f16, tag=f"yt{i}", name=f"yt{i}") for i in range(4)]
            va_sb = [qkvp.tile([128, 8, 65], f16, tag=f"va{i}", name=f"va{i}") for i in range(NS)]

            from concourse.tile import add_dep_helper

            pend = []

            def _flush_tail(item, anchor):
                """Emit the deferred softmax-divide tail. `anchor` is a PE
                instruction the broadcast matmul is ordered after, giving the
                DVE reciprocal time to finish without stalling the PE."""
                rec, yr, hp_, qrow_, tcx_ = item
                bps = psA.tile([64, 512], f32, tag="psA", name="bps")
                mm = nc.tensor.matmul(
                    bps[:], ones_sb[:, 0:64], rec[:], start=True, stop=True
                )
                if anchor is not None:
                    add_dep_helper(
                        mm.ins, anchor.ins, reason="defer tail past anchor"
                    )
                bcs = stgp.tile([64, 512], f32, tag="bcs", bufs=3, name="bcs")
                nc.vector.tensor_copy(bcs[:], bps[:])
                nc.vector.tensor_mul(
                    yt_sb[hp_][qrow_, tcx_ * 512:(tcx_ + 1) * 512],
                    yr[0:64, :],
                    bcs[:],
                )

            # t-chunk-outer structure: projections for chunk tcx, then
            # attention for all heads at tcx (keys/values <= tcx are ready),
            # then the output projection for tcx's t-tiles.  The scheduler
            # can interleave across sections to keep the PE stream dense.
            def _oproj(tcx):
                for tt in range(4 * tcx, 4 * tcx + 4):
                    for cc in range(2):
                        ps = psA.tile([128, 512], f32, tag="psA", name="ops")
                        for hp in range(4):
                            nc.tensor.matmul(
                                ps[:],
                                yt_sb[hp][:, tt * 128:(tt + 1) * 128],
                                wo_sb[hp][:, cc * 512:(cc + 1) * 512],
                                start=(hp == 0),
                                stop=(hp == 3),
                            )
                        osb = outp.tile([128, 512], f32, tag="osb", name="osb")
                        nc.vector.tensor_copy(osb[:], ps[:])
                        nc.sync.dma_start(
                            out[tt * 128:(tt + 1) * 128,
                                cc * 512:(cc + 1) * 512],
                            osb[:],
                        )

            for tcx in range(NT):
                # -- qT/kT projection for this t-chunk --
                anchor_mid = None
                for jt in range(8):
                    dest = qt_sb[jt] if jt < 4 else kt_sb[jt - 4]
                    ps = psA.tile([128, 512], f32, tag="psA")
                    for et in range(NE):
                        mm = nc.tensor.matmul(
                            ps[:],
                            wqk_sb[et][:, jt * 128:(jt + 1) * 128],
                            xt_sb[et][:, tcx * 512:(tcx + 1) * 512],
                            start=(et == 0),
                            stop=(et == NE - 1),
                        )
                    if jt == 3:
                        anchor_mid = mm
                    nc.vector.tensor_scalar_add(
                        dest[:, tcx * 512:(tcx + 1) * 512], ps[:],
                        bqk_sb[:, jt:jt + 1],
                    )

                # previous chunk's last head tail + output projection, placed
                # here so its reciprocal hides under this chunk's projections
                while pend:
                    _flush_tail(pend.pop(0), anchor_mid)
                if tcx > 0:
                    _oproj(tcx - 1)

                # -- V projection for this chunk's 4 s-tiles --
                for st in range(4 * tcx, 4 * tcx + 4):
                    ps = psA.tile([128, 512], f32, tag="psA")
                    for et in range(NE):
                        nc.tensor.matmul(
                            ps[:],
                            xt_sb[et][:, st * 128:(st + 1) * 128],
                            wv_sb[et][:],
                            start=(et == 0),
                            stop=False,
                        )
                    # bias row: V += 1 * bv
                    nc.tensor.matmul(
                        ps[:], ones_sb[:], bv_sb[:], start=False, stop=True,
                    )
                    va = va_sb[st]
                    nc.vector.tensor_copy(
                        va[:, :, 0:64],
                        ps[:].rearrange("p (h c) -> p h c", c=64),
                    )
                    nc.vector.memset(va[:, :, 64:65], 1.0)

                # -- attention for all heads at this t-chunk --
                # The per-head tail (reciprocal -> broadcast-matmul -> divide)
                # is deferred into the middle of the NEXT head's matmul stream
                # so the in-order PE never stalls on the 3.3us DVE reciprocal.
                nst = 4 * (tcx + 1)
                for h in range(HPC):
                    hp, ho = divmod(h, 2)
                    qrow = slice(ho * 64, (ho + 1) * 64)
                    yps = psY.tile([65, 512], f32, tag="psY")
                    for g in range(0, nst, 4):
                        gn = min(4, nst - g)
                        sgrp = []
                        for st in range(g, g + gn):
                            diag = st * 128 >= tcx * 512
                            sps = psS.tile([128, 512], f32, tag="psS")
                            nc.tensor.matmul(
                                sps[:],
                                kt_sb[hp][qrow, st * 128:(st + 1) * 128],
                                qt_sb[hp][qrow, tcx * 512:(tcx + 1) * 512],
                                start=True,
                                stop=not diag,
                            )
                            lo = max(0, st * 128 - tcx * 512)
                            if diag:
                                # additive causal mask on the diagonal window
                                nc.tensor.matmul(
                                    sps[:, lo:lo + 128],
                                    ident_sb[:],
                                    mask_sb[:],
                                    start=False,
                                    stop=True,
                                )
                            pt = ptp.tile([128, 512], f16, tag="pt")
                            nc.scalar.activation(
                                pt[:, lo:512], sps[:, lo:512], AF.Exp
                            )
                            sgrp.append((st, lo, pt))
                        av_last = None
                        for st, lo, pt in sgrp:
                            av_last = nc.tensor.matmul(
                                yps[:, lo:512],
                                va_sb[st][:, h, :],
                                pt[:, lo:512],
                                start=(st == 0),
                                stop=(st == nst - 1),
                            )
                        if pend and (g == 4 or (g == 0 and nst == 4)):
                            _flush_tail(pend.pop(0), av_last)
                    # evacuate the accumulator to SBUF on ACT so the PSUM bank
                    # frees immediately; reciprocal + divide run from SBUF
                    yr = stgp.tile([65, 512], f32, tag="yr", bufs=3, name="yr")
                    nc.scalar.activation(yr[:], yps[:], AF.Copy)
                    rec = recp.tile([1, 512], f16, tag="rec", bufs=3)
                    with nc.allow_low_precision(reason="fp16 softmax recip"):
                        nc.vector.reciprocal(rec[:], yr[64:65, :])
                    pend.append((rec, yr, hp, qrow, tcx))

            # final chunk: remaining tails (short PE stall) + last oproj
            while pend:
                _flush_tail(pend.pop(0), None)
            _oproj(NT - 1)
    return _split_excess_waits(nc, max_waits=1)


def _prep_in_maps(x, W_qkv, b_qkv, W_out):
    f16 = np.float16
    x = np.asarray(x, np.float32)
    W_qkv = np.asarray(W_qkv, np.float32)
    b_qkv = np.asarray(b_qkv, np.float32)
    W_out = np.asarray(W_out, np.float32)

    mask = np.where(
        np.triu(np.ones((128, 128), dtype=bool)), 0.0, -30000.0
    ).astype(f16)
    ident = np.eye(128, dtype=f16)
    ones1 = np.ones((1, 128), dtype=f16)
    in_maps = []
    for c in range(8):
        b, hg = divmod(c, 2)
        qs = slice(hg * 512, (hg + 1) * 512)
        ks = slice(E + hg * 512, E + (hg + 1) * 512)
        vs = slice(2 * E + hg * 512, 2 * E + (hg + 1) * 512)
        wqk_c = np.concatenate(
            [W_qkv[:, qs] * 0.125, W_qkv[:, ks]], axis=1
        ).astype(f16)
        bqk_c = np.concatenate(
            [b_qkv[qs] * 0.125, b_qkv[ks]]
        ).astype(np.float32).reshape(8, 128).T.copy()
        in_maps.append({
            "xt": np.ascontiguousarray(x[b].T).astype(f16),
            "ident": ident,
            "wqk": wqk_c,
            "bqk": bqk_c,
            "wv": W_qkv[:, vs].astype(f16),
            "bv": b_qkv[vs].astype(f16).reshape(1, 512),
            "wo": W_out[hg * 512:(hg + 1) * 512, :].astype(f16),
            "mask": mask,
            "ones1": ones1,
        })
    return in_maps


def run(x, W_qkv, b_qkv, W_out, b_out, trace=False, **trace_kwargs):
    from concourse.bass_utils import run_bass_kernel_spmd

    if "nc" not in _CACHE:
        _CACHE["nc"] = _build()
    nc = _CACHE["nc"]
    in_maps = _prep_in_maps(x, W_qkv, b_qkv, W_out)
    res = run_bass_kernel_spmd(
        nc, in_maps, list(range(8)), trace=trace, **trace_kwargs
    )
    parts = [r["out"] for r in res.results]
    b_out = np.asarray(b_out, np.float32)
    y = np.stack([parts[2 * b] + parts[2 * b + 1] for b in range(B)]) + b_out
    return y.astype(np.float32), res


def kernel(x, W_qkv, b_qkv, W_out, b_out):
    y, _ = run(x, W_qkv, b_qkv, W_out, b_out, trace=False)
    return y


# revision 45
# speedup vs baseline: 1.2395x; 1.0089x over previous
"""Causal self-attention on 8 trn2 NeuronCores.

Sharding: core c -> (batch b = c//2, head-group hg = c%2 of 8 heads).
Each core computes, for its batch and its 8 heads:
  qT,kT = (x[b] @ Wqk_shard).T        (q pre-scaled by 1/sqrt(hd))
  V     = x[b] @ Wv_shard
  S^T   = kT_h.T @ qT_h  per head     (s on partitions, t on free dim)
  P^T   = exp(S^T) with causal mask   (no max-subtraction: logits are O(5))
  yT    = V_aug.T @ P^T               (V carries a ones column -> row 64 = softmax denom)
  out_partial = y_local @ Wout_rows   ([T, E] fp32 partial sum)
Host: out[b] = partial[2b] + partial[2b+1] + b_out.

All matmul inputs fp16, PSUM accumulation fp32. x is pre-transposed and
pre-cast on host so no on-chip transpose is needed.
"""

import numpy as np

B, T, E, H, HD = 4, 2048, 1024, 16, 64
HPC = 8            # heads per core
DL = HPC * HD      # 512 local y dims per core
NT = T // 512      # 4 t-chunks of 512
NS = T // 128      # 16 s-tiles of 128
NE = E // 128      # 8 e-tiles

_CACHE = {}


def _make_tc_class():
    """TileContext whose tail drain splits sem waits across single-wait NOPs.

    The walrus build in this container rejects instructions carrying more
    than a couple of sync waits ("Too many sync wait commands" on the Tile
    tail Drain), so emit one NOP per logical proc, each with one wait.
    """
    import concourse.tile as tile
    from concourse.vector_clock import ScopedClock, VectorClock

    class TC(tile.TileContext):
        def _drain_and_barrier(self, tick_clock, wait_clock):
            gc = tick_clock.global_clock
            n = len(gc)
            for i in range(n):
                if gc[i] > 0:
                    vc = VectorClock([0] * n)
                    vc.require_at_least(i, gc[i])
                    nop = self.nc.sync.nop(nofuse=True)
                    wait_clock.add_sem_waits(nop.ins, ScopedClock({None: vc}))
            self.nc.sync.drain()
            self.nc.all_engine_barrier()
            assert self.sems is not None
            popped = self.nc._tile_sem_poison_stack.pop()
            assert popped is self._sem_poison
            self.nc.clear_and_free_semaphores(
                list(self.sems.allocated().values())
            )
            self.nc.all_engine_barrier()

    return TC


def _split_excess_waits(nc, max_waits=2):
    """Walrus in this container caps sem waits per instruction; hoist any
    excess waits onto fresh same-engine NOPs inserted just before."""
    import concourse.mybir as mybir

    n = 0
    for f in nc.m.functions:
        for bb in f.blocks:
            insts = bb.instructions
            out = []
            for inst in insts:
                si = inst.sync_info
                if si is not None and len(si.on_wait) > max_waits:
                    w = list(si.on_wait)
                    excess, keep = w[:-max_waits], w[-max_waits:]
                    for k in range(0, len(excess), max_waits):
                        nop = mybir.InstNoOp(
                            name=f"I-splitw-{n}", ins=[], outs=[]
                        )
                        n += 1
                        nop.engine = inst.engine
                        nop.sync_info = mybir.SyncInfo(
                            on_wait=excess[k:k + max_waits], on_update=[]
                        )
                        out.append(nop)
                    inst.sync_info = mybir.SyncInfo(
                        on_wait=keep, on_update=si.on_update
                    )
                out.append(inst)
            if n:
                bb.instructions = out
    return nc


def _build():
    import concourse.bass as bass
    import concourse.mybir as mybir

    dt = mybir.dt
    f16, f32 = dt.float16, dt.float32
    AF = mybir.ActivationFunctionType

    nc = bass.Bass()
    xt = nc.declare_dram_parameter("xt", [E, T], f16, isOutput=False)
    wqk = nc.declare_dram_parameter("wqk", [E, 1024], f16, isOutput=False)
    bqk = nc.declare_dram_parameter("bqk", [128, 8], f32, isOutput=False)
    wv = nc.declare_dram_parameter("wv", [E, 512], f16, isOutput=False)
    bv = nc.declare_dram_parameter("bv", [1, 512], f16, isOutput=False)
    wo = nc.declare_dram_parameter("wo", [DL, E], f16, isOutput=False)
    # maskadd[i, j] = 0 if j >= i else -30000 (additive causal mask)
    mask = nc.declare_dram_parameter("mask", [128, 128], f16, isOutput=False)
    ident = nc.declare_dram_parameter("ident", [128, 128], f16, isOutput=False)
    ones1 = nc.declare_dram_parameter("ones1", [1, 128], f16, isOutput=False)
    out = nc.declare_dram_parameter("out", [T, E], f32, isOutput=True)

    with _make_tc_class()(nc) as tc:
        with (
            tc.tile_pool(name="const", bufs=1) as constp,
            tc.tile_pool(name="xtp", bufs=1) as xtp,
            tc.tile_pool(name="wp", bufs=1) as wp,
            tc.tile_pool(name="qkv", bufs=1) as qkvp,
            tc.tile_pool(name="pt", bufs=6) as ptp,
            tc.tile_pool(name="rec", bufs=2) as recp,
            tc.tile_pool(name="stg", bufs=2) as stgp,
            tc.tile_pool(name="outp", bufs=3) as outp,
            tc.tile_pool(name="psA", bufs=2, space="PSUM") as psA,
            tc.tile_pool(name="psS", bufs=4, space="PSUM") as psS,
            tc.tile_pool(name="psY", bufs=2, space="PSUM") as psY,
        ):
            # ---- constants / weights ----
            bqk_sb = constp.tile([128, 8], f32, tag="bqk")
            nc.sync.dma_start(bqk_sb[:], bqk[:])
            bv_sb = constp.tile([1, 512], f16, tag="bv")
            nc.sync.dma_start(bv_sb[:], bv[:])
            mask_sb = constp.tile([128, 128], f16, tag="mask")
            nc.sync.dma_start(mask_sb[:], mask[:])
            ones_sb = constp.tile([1, 128], f16, tag="ones1")
            nc.sync.dma_start(ones_sb[:], ones1[:])
            ident_sb = constp.tile([128, 128], f16, tag="ident")
            nc.sync.dma_start(ident_sb[:], ident[:])

            # interleave x / weight loads so the first projection matmul
            # (needs wqk[0] + xt[0]) can start after the first two transfers
            xt_sb, wqk_sb, wv_sb, wo_sb = [], [], [], []
            for i in range(NE):
                w_ = wp.tile([128, 1024], f16, tag=f"wqk{i}", name=f"wqk{i}")
                nc.sync.dma_start(w_[:], wqk[i * 128:(i + 1) * 128, :])
                wqk_sb.append(w_)
                t_ = xtp.tile([128, T], f16, tag=f"xt{i}", name=f"xt{i}")
                nc.sync.dma_start(t_[:], xt[i * 128:(i + 1) * 128, :])
                xt_sb.append(t_)
            for i in range(NE):
                t_ = wp.tile([128, 512], f16, tag=f"wv{i}", name=f"wv{i}")
                nc.sync.dma_start(t_[:], wv[i * 128:(i + 1) * 128, :])
                wv_sb.append(t_)
            for i in range(4):
                t_ = wp.tile([128, 1024], f16, tag=f"wo{i}", name=f"wo{i}")
                nc.sync.dma_start(t_[:], wo[i * 128:(i + 1) * 128, :])
                wo_sb.append(t_)

            qt_sb = [qkvp.tile([128, T], f16, tag=f"qt{i}", name=f"qt{i}") for i in range(4)]
            kt_sb = [qkvp.tile([128, T], f16, tag=f"kt{i}", name=f"kt{i}") for i in range(4)]
            yt_sb = [qkvp.tile([128, T], f16, tag=f"yt{i}", name=f"yt{i}") for i in range(4)]
            va_sb = [qkvp.tile([128, 8, 65], f16, tag=f"va{i}", name=f"va{i}") for i in range(NS)]

            from concourse.tile import add_dep_helper

            pend = []

            def _flush_tail(item, anchor):
                """Emit the deferred softmax-divide tail. `anchor` is a PE
                instruction the broadcast matmul is ordered after, giving the
                DVE reciprocal time to finish without stalling the PE."""
                rec, yr, hp_, qrow_, tcx_ = item
                bps = psA.tile([64, 512], f32, tag="psA", name="bps")
                mm = nc.tensor.matmul(
                    bps[:], ones_sb[:, 0:64], rec[:], start=True, stop=True
                )
                if anchor is not None:
                    add_dep_helper(
                        mm.ins, anchor.ins, reason="defer tail past anchor"
                    )
                bcs = stgp.tile([64, 512], f32, tag="bcs", bufs=3, name="bcs")
                nc.vector.tensor_copy(bcs[:], bps[:])
                nc.vector.tensor_mul(
                    yt_sb[hp_][qrow_, tcx_ * 512:(tcx_ + 1) * 512],
                    yr[0:64, :],
                    bcs[:],
                )

            # t-chunk-outer structure: projections for chunk tcx, then
            # attention for all heads at tcx (keys/values <= tcx are ready),
            # then the output projection for tcx's t-tiles.  The scheduler
            # can interleave across sections to keep the PE stream dense.
            def _oproj(tcx):
                for tt in range(4 * tcx, 4 * tcx + 4):
                    for cc in range(2):
                        ps = psA.tile([128, 512], f32, tag="psA", name="ops")
                        for hp in range(4):
                            nc.tensor.matmul(
                                ps[:],
                                yt_sb[hp][:, tt * 128:(tt + 1) * 128],
                                wo_sb[hp][:, cc * 512:(cc + 1) * 512],
                                start=(hp == 0),
                                stop=(hp == 3),
                            )
                        osb = outp.tile([128, 512], f32, tag="osb", name="osb")
                        nc.vector.tensor_copy(osb[:], ps[:])
                        nc.sync.dma_start(
                            out[tt * 128:(tt + 1) * 128,
                                cc * 512:(cc + 1) * 512],
                            osb[:],
                        )

            for tcx in range(NT):
                # -- qT/kT projection for this t-chunk --
                anchor_mid = None
                for jt in range(8):
                    dest = qt_sb[jt] if jt < 4 else kt_sb[jt - 4]
                    ps = psA.tile([128, 512], f32, tag="psA")
                    for et in range(NE):
                        mm = nc.tensor.matmul(
                            ps[:],
                            wqk_sb[et][:, jt * 128:(jt + 1) * 128],
                            xt_sb[et][:, tcx * 512:(tcx + 1) * 512],
                            start=(et == 0),
                            stop=(et == NE - 1),
                        )
                    if jt == 3:
                        anchor_mid = mm
                    nc.vector.tensor_scalar_add(
                        dest[:, tcx * 512:(tcx + 1) * 512], ps[:],
                        bqk_sb[:, jt:jt + 1],
                    )

                # previous chunk's last head tail + output projection, placed
                # here so its reciprocal hides under this chunk's projections
                while pend:
                    _flush_tail(pend.pop(0), anchor_mid)
                if tcx > 0:
                    _oproj(tcx - 1)

                # -- V projection for this chunk's 4 s-tiles --
                for st in range(4 * tcx, 4 * tcx + 4):
                    ps = psA.tile([128, 512], f32, tag="psA")
                    for et in range(NE):
                        nc.tensor.matmul(
                            ps[:],
                            xt_sb[et][:, st * 128:(st + 1) * 128],
                            wv_sb[et][:],
                            start=(et == 0),
                            stop=False,
                        )
                    # bias row: V += 1 * bv
                    nc.tensor.matmul(
                        ps[:], ones_sb[:], bv_sb[:], start=False, stop=True,
                    )
                    va = va_sb[st]
                    nc.vector.tensor_copy(
                        va[:, :, 0:64],
                        ps[:].rearrange("p (h c) -> p h c", c=64),
                    )
                    nc.vector.memset(va[:, :, 64:65], 1.0)

                # -- attention for all heads at this t-chunk --
                # The per-head tail (reciprocal -> broadcast-matmul -> divide)
                # is deferred into the middle of the NEXT head's matmul stream
                # so the in-order PE never stalls on the 3.3us DVE reciprocal.
                nst = 4 * (tcx + 1)
                for h in range(HPC):
                    hp, ho = divmod(h, 2)
                    qrow = slice(ho * 64, (ho + 1) * 64)
                    yps = psY.tile([65, 512], f32, tag="psY")
                    for g in range(0, nst, 4):
                        gn = min(4, nst - g)
                        sgrp = []
                        for st in range(g, g + gn):
                            diag = st * 128 >= tcx * 512
                            sps = psS.tile([128, 512], f32, tag="psS")
                            nc.tensor.matmul(
                                sps[:],
                                kt_sb[hp][qrow, st * 128:(st + 1) * 128],
                                qt_sb[hp][qrow, tcx * 512:(tcx + 1) * 512],
                                start=True,
                                stop=not diag,
                            )
                            lo = max(0, st * 128 - tcx * 512)
                            if diag:
                                # additive causal mask on the diagonal window
                                nc.tensor.matmul(
                                    sps[:, lo:lo + 128],
                                    ident_sb[:],
                                    mask_sb[:],
                                    start=False,
                                    stop=True,
                                )
                            pt = ptp.tile([128, 512], f16, tag="pt")
                            nc.scalar.activation(
                                pt[:, lo:512], sps[:, lo:512], AF.Exp
                            )
                            sgrp.append((st, lo, pt))
                        av_last = None
                        for st, lo, pt in sgrp:
                            av_last = nc.tensor.matmul(
                                yps[:, lo:512],
                                va_sb[st][:, h, :],
                                pt[:, lo:512],
                                start=(st == 0),
                                stop=(st == nst - 1),
                            )
                        if pend and (
                            g == 4
                            or (g == 0 and nst == 4 and len(pend) >= 2)
                        ):
                            _flush_tail(pend.pop(0), av_last)
                    # evacuate the accumulator to SBUF on ACT so the PSUM bank
                    # frees immediately; reciprocal + divide run from SBUF
                    yr = stgp.tile([65, 512], f32, tag="yr", bufs=3, name="yr")
                    nc.scalar.activation(yr[:], yps[:], AF.Copy)
                    rec = recp.tile([1, 512], f16, tag="rec", bufs=3)
                    with nc.allow_low_precision(reason="fp16 softmax recip"):
                        nc.vector.reciprocal(rec[:], yr[64:65, :])
                    pend.append((rec, yr, hp, qrow, tcx))

            # final chunk: remaining tails (short PE stall) + last oproj
            while pend:
                _flush_tail(pend.pop(0), None)
            _oproj(NT - 1)
    return _split_excess_waits(nc, max_waits=1)


def _prep_in_maps(x, W_qkv, b_qkv, W_out):
    f16 = np.float16
    x = np.asarray(x, np.float32)
    W_qkv = np.asarray(W_qkv, np.float32)
    b_qkv = np.asarray(b_qkv, np.float32)
    W_out = np.asarray(W_out, np.float32)

    mask = np.where(
        np.triu(np.ones((128, 128), dtype=bool)), 0.0, -30000.0
    ).astype(f16)
    ident = np.eye(128, dtype=f16)
    ones1 = np.ones((1, 128), dtype=f16)
    in_maps = []
    for c in range(8):
        b, hg = divmod(c, 2)
        qs = slice(hg * 512, (hg + 1) * 512)
        ks = slice(E + hg * 512, E + (hg + 1) * 512)
        vs = slice(2 * E + hg * 512, 2 * E + (hg + 1) * 512)
        wqk_c = np.concatenate(
            [W_qkv[:, qs] * 0.125, W_qkv[:, ks]], axis=1
        ).astype(f16)
        bqk_c = np.concatenate(
            [b_qkv[qs] * 0.125, b_qkv[ks]]
        ).astype(np.float32).reshape(8, 128).T.copy()
        in_maps.append({
            "xt": np.ascontiguousarray(x[b].T).astype(f16),
            "ident": ident,
            "wqk": wqk_c,
            "bqk": bqk_c,
            "wv": W_qkv[:, vs].astype(f16),
            "bv": b_qkv[vs].astype(f16).reshape(1, 512),
            "wo": W_out[hg * 512:(hg + 1) * 512, :].astype(f16),
            "mask": mask,
            "ones1": ones1,
        })
    return in_maps


def run(x, W_qkv, b_qkv, W_out, b_out, trace=False, **trace_kwargs):
    from concourse.bass_utils import run_bass_kernel_spmd

    if "nc" not in _CACHE:
        _CACHE["nc"] = _build()
    nc = _CACHE["nc"]
    in_maps = _prep_in_maps(x, W_qkv, b_qkv, W_out)
    res = run_bass_kernel_spmd(
        nc, in_maps, list(range(8)), trace=trace, **trace_kwargs
    )
    parts = [r["out"] for r in res.results]
    b_out = np.asarray(b_out, np.float32)
    y = np.stack([parts[2 * b] + parts[2 * b + 1] for b in range(B)]) + b_out
    return y.astype(np.float32), res


def kernel(x, W_qkv, b_qkv, W_out, b_out):
    y, _ = run(x, W_qkv, b_qkv, W_out, b_out, trace=False)
    return y


# revision 46
# speedup vs baseline: 1.2687x; 1.0236x over previous
"""Causal self-attention on 8 trn2 NeuronCores.

Sharding: core c -> (batch b = c//2, head-group hg = c%2 of 8 heads).
Each core computes, for its batch and its 8 heads:
  qT,kT = (x[b] @ Wqk_shard).T        (q pre-scaled by 1/sqrt(hd))
  V     = x[b] @ Wv_shard
  S^T   = kT_h.T @ qT_h  per head     (s on partitions, t on free dim)
  P^T   = exp(S^T) with causal mask   (no max-subtraction: logits are O(5))
  yT    = V_aug.T @ P^T               (V carries a ones column -> row 64 = softmax denom)
  out_partial = y_local @ Wout_rows   ([T, E] fp32 partial sum)
Host: out[b] = partial[2b] + partial[2b+1] + b_out.

All matmul inputs fp16, PSUM accumulation fp32. x is pre-transposed and
pre-cast on host so no on-chip transpose is needed.
"""

import numpy as np

B, T, E, H, HD = 4, 2048, 1024, 16, 64
HPC = 8            # heads per core
DL = HPC * HD      # 512 local y dims per core
NT = T // 512      # 4 t-chunks of 512
NS = T // 128      # 16 s-tiles of 128
NE = E // 128      # 8 e-tiles

_CACHE = {}


def _make_tc_class():
    """TileContext whose tail drain splits sem waits across single-wait NOPs.

    The walrus build in this container rejects instructions carrying more
    than a couple of sync waits ("Too many sync wait commands" on the Tile
    tail Drain), so emit one NOP per logical proc, each with one wait.
    """
    import concourse.tile as tile
    from concourse.vector_clock import ScopedClock, VectorClock

    class TC(tile.TileContext):
        def _drain_and_barrier(self, tick_clock, wait_clock):
            gc = tick_clock.global_clock
            n = len(gc)
            for i in range(n):
                if gc[i] > 0:
                    vc = VectorClock([0] * n)
                    vc.require_at_least(i, gc[i])
                    nop = self.nc.sync.nop(nofuse=True)
                    wait_clock.add_sem_waits(nop.ins, ScopedClock({None: vc}))
            self.nc.sync.drain()
            self.nc.all_engine_barrier()
            assert self.sems is not None
            popped = self.nc._tile_sem_poison_stack.pop()
            assert popped is self._sem_poison
            self.nc.clear_and_free_semaphores(
                list(self.sems.allocated().values())
            )
            self.nc.all_engine_barrier()

    return TC


def _split_excess_waits(nc, max_waits=2):
    """Walrus in this container caps sem waits per instruction; hoist any
    excess waits onto fresh same-engine NOPs inserted just before."""
    import concourse.mybir as mybir

    n = 0
    for f in nc.m.functions:
        for bb in f.blocks:
            insts = bb.instructions
            out = []
            for inst in insts:
                si = inst.sync_info
                if si is not None and len(si.on_wait) > max_waits:
                    w = list(si.on_wait)
                    excess, keep = w[:-max_waits], w[-max_waits:]
                    for k in range(0, len(excess), max_waits):
                        nop = mybir.InstNoOp(
                            name=f"I-splitw-{n}", ins=[], outs=[]
                        )
                        n += 1
                        nop.engine = inst.engine
                        nop.sync_info = mybir.SyncInfo(
                            on_wait=excess[k:k + max_waits], on_update=[]
                        )
                        out.append(nop)
                    inst.sync_info = mybir.SyncInfo(
                        on_wait=keep, on_update=si.on_update
                    )
                out.append(inst)
            if n:
                bb.instructions = out
    return nc


def _build():
    import concourse.bass as bass
    import concourse.mybir as mybir

    dt = mybir.dt
    f16, f32 = dt.float16, dt.float32
    AF = mybir.ActivationFunctionType

    nc = bass.Bass()
    xt = nc.declare_dram_parameter("xt", [E, T], f16, isOutput=False)
    wqk = nc.declare_dram_parameter("wqk", [E, 1024], f16, isOutput=False)
    bqk = nc.declare_dram_parameter("bqk", [128, 8], f32, isOutput=False)
    wv = nc.declare_dram_parameter("wv", [E, 512], f16, isOutput=False)
    bv = nc.declare_dram_parameter("bv", [1, 512], f16, isOutput=False)
    wo = nc.declare_dram_parameter("wo", [DL, E], f16, isOutput=False)
    # maskadd[i, j] = 0 if j >= i else -30000 (additive causal mask)
    mask = nc.declare_dram_parameter("mask", [128, 128], f16, isOutput=False)
    ident = nc.declare_dram_parameter("ident", [128, 128], f16, isOutput=False)
    ones1 = nc.declare_dram_parameter("ones1", [1, 128], f16, isOutput=False)
    out = nc.declare_dram_parameter("out", [T, E], f32, isOutput=True)

    with _make_tc_class()(nc) as tc:
        with (
            tc.tile_pool(name="const", bufs=1) as constp,
            tc.tile_pool(name="xtp", bufs=1) as xtp,
            tc.tile_pool(name="wp", bufs=1) as wp,
            tc.tile_pool(name="qkv", bufs=1) as qkvp,
            tc.tile_pool(name="pt", bufs=6) as ptp,
            tc.tile_pool(name="rec", bufs=2) as recp,
            tc.tile_pool(name="stg", bufs=2) as stgp,
            tc.tile_pool(name="outp", bufs=3) as outp,
            tc.tile_pool(name="psA", bufs=3, space="PSUM") as psA,
            tc.tile_pool(name="psS", bufs=3, space="PSUM") as psS,
            tc.tile_pool(name="psY", bufs=2, space="PSUM") as psY,
        ):
            # ---- constants / weights ----
            bqk_sb = constp.tile([128, 8], f32, tag="bqk")
            nc.sync.dma_start(bqk_sb[:], bqk[:])
            bv_sb = constp.tile([1, 512], f16, tag="bv")
            nc.sync.dma_start(bv_sb[:], bv[:])
            mask_sb = constp.tile([128, 128], f16, tag="mask")
            nc.sync.dma_start(mask_sb[:], mask[:])
            ones_sb = constp.tile([1, 128], f16, tag="ones1")
            nc.sync.dma_start(ones_sb[:], ones1[:])
            ident_sb = constp.tile([128, 128], f16, tag="ident")
            nc.sync.dma_start(ident_sb[:], ident[:])

            # interleave x / weight loads so the first projection matmul
            # (needs wqk[0] + xt[0]) can start after the first two transfers
            xt_sb, wqk_sb, wv_sb, wo_sb = [], [], [], []
            for i in range(NE):
                w_ = wp.tile([128, 1024], f16, tag=f"wqk{i}", name=f"wqk{i}")
                nc.sync.dma_start(w_[:], wqk[i * 128:(i + 1) * 128, :])
                wqk_sb.append(w_)
                t_ = xtp.tile([128, T], f16, tag=f"xt{i}", name=f"xt{i}")
                nc.sync.dma_start(t_[:], xt[i * 128:(i + 1) * 128, :])
                xt_sb.append(t_)
            for i in range(NE):
                t_ = wp.tile([128, 512], f16, tag=f"wv{i}", name=f"wv{i}")
                nc.sync.dma_start(t_[:], wv[i * 128:(i + 1) * 128, :])
                wv_sb.append(t_)
            for i in range(4):
                t_ = wp.tile([128, 1024], f16, tag=f"wo{i}", name=f"wo{i}")
                nc.sync.dma_start(t_[:], wo[i * 128:(i + 1) * 128, :])
                wo_sb.append(t_)

            qt_sb = [qkvp.tile([128, T], f16, tag=f"qt{i}", name=f"qt{i}") for i in range(4)]
            kt_sb = [qkvp.tile([128, T], f16, tag=f"kt{i}", name=f"kt{i}") for i in range(4)]
            yt_sb = [qkvp.tile([128, T], f16, tag=f"yt{i}", name=f"yt{i}") for i in range(4)]
            va_sb = [qkvp.tile([128, 8, 65], f16, tag=f"va{i}", name=f"va{i}") for i in range(NS)]

            from concourse.tile import add_dep_helper

            pend = []

            def _flush_tail(item, anchor):
                """Emit the deferred softmax-divide tail. `anchor` is a PE
                instruction the broadcast matmul is ordered after, giving the
                DVE reciprocal time to finish without stalling the PE."""
                rec, yr, hp_, qrow_, tcx_ = item
                bps = psA.tile([64, 512], f32, tag="psA", name="bps")
                mm = nc.tensor.matmul(
                    bps[:], ones_sb[:, 0:64], rec[:], start=True, stop=True
                )
                if anchor is not None:
                    add_dep_helper(
                        mm.ins, anchor.ins, reason="defer tail past anchor"
                    )
                bcs = stgp.tile([64, 512], f32, tag="bcs", bufs=3, name="bcs")
                nc.vector.tensor_copy(bcs[:], bps[:])
                nc.vector.tensor_mul(
                    yt_sb[hp_][qrow_, tcx_ * 512:(tcx_ + 1) * 512],
                    yr[0:64, :],
                    bcs[:],
                )

            # t-chunk-outer structure: projections for chunk tcx, then
            # attention for all heads at tcx (keys/values <= tcx are ready),
            # then the output projection for tcx's t-tiles.  The scheduler
            # can interleave across sections to keep the PE stream dense.
            def _oproj(tcx):
                for tt in range(4 * tcx, 4 * tcx + 4):
                    for cc in range(2):
                        ps = psA.tile([128, 512], f32, tag="psA", name="ops")
                        for hp in range(4):
                            nc.tensor.matmul(
                                ps[:],
                                yt_sb[hp][:, tt * 128:(tt + 1) * 128],
                                wo_sb[hp][:, cc * 512:(cc + 1) * 512],
                                start=(hp == 0),
                                stop=(hp == 3),
                            )
                        osb = outp.tile([128, 512], f32, tag="osb", name="osb")
                        nc.vector.tensor_copy(osb[:], ps[:])
                        nc.sync.dma_start(
                            out[tt * 128:(tt + 1) * 128,
                                cc * 512:(cc + 1) * 512],
                            osb[:],
                        )

            for tcx in range(NT):
                # -- qT/kT projection for this t-chunk --
                anchor_mid = None
                for jt in range(8):
                    dest = qt_sb[jt] if jt < 4 else kt_sb[jt - 4]
                    ps = psA.tile([128, 512], f32, tag="psA")
                    for et in range(NE):
                        mm = nc.tensor.matmul(
                            ps[:],
                            wqk_sb[et][:, jt * 128:(jt + 1) * 128],
                            xt_sb[et][:, tcx * 512:(tcx + 1) * 512],
                            start=(et == 0),
                            stop=(et == NE - 1),
                        )
                    if jt == 3:
                        anchor_mid = mm
                    nc.vector.tensor_scalar_add(
                        dest[:, tcx * 512:(tcx + 1) * 512], ps[:],
                        bqk_sb[:, jt:jt + 1],
                    )

                # previous chunk's last head tail + output projection, placed
                # here so its reciprocal hides under this chunk's projections
                while pend:
                    _flush_tail(pend.pop(0), anchor_mid)
                if tcx > 0:
                    _oproj(tcx - 1)

                # -- V projection for this chunk's 4 s-tiles --
                for st in range(4 * tcx, 4 * tcx + 4):
                    ps = psA.tile([128, 512], f32, tag="psA")
                    for et in range(NE):
                        nc.tensor.matmul(
                            ps[:],
                            xt_sb[et][:, st * 128:(st + 1) * 128],
                            wv_sb[et][:],
                            start=(et == 0),
                            stop=False,
                        )
                    # bias row: V += 1 * bv
                    nc.tensor.matmul(
                        ps[:], ones_sb[:], bv_sb[:], start=False, stop=True,
                    )
                    va = va_sb[st]
                    nc.vector.tensor_copy(
                        va[:, :, 0:64],
                        ps[:].rearrange("p (h c) -> p h c", c=64),
                    )
                    nc.vector.memset(va[:, :, 64:65], 1.0)

                # -- attention for all heads at this t-chunk --
                # The per-head tail (reciprocal -> broadcast-matmul -> divide)
                # is deferred into the middle of the NEXT head's matmul stream
                # so the in-order PE never stalls on the 3.3us DVE reciprocal.
                nst = 4 * (tcx + 1)
                for h in range(HPC):
                    hp, ho = divmod(h, 2)
                    qrow = slice(ho * 64, (ho + 1) * 64)
                    yps = psY.tile([65, 512], f32, tag="psY")
                    for g in range(0, nst, 4):
                        gn = min(4, nst - g)
                        sgrp = []
                        for st in range(g, g + gn):
                            diag = st * 128 >= tcx * 512
                            sps = psS.tile([128, 512], f32, tag="psS")
                            nc.tensor.matmul(
                                sps[:],
                                kt_sb[hp][qrow, st * 128:(st + 1) * 128],
                                qt_sb[hp][qrow, tcx * 512:(tcx + 1) * 512],
                                start=True,
                                stop=not diag,
                            )
                            lo = max(0, st * 128 - tcx * 512)
                            if diag:
                                # additive causal mask on the diagonal window
                                nc.tensor.matmul(
                                    sps[:, lo:lo + 128],
                                    ident_sb[:],
                                    mask_sb[:],
                                    start=False,
                                    stop=True,
                                )
                            pt = ptp.tile([128, 512], f16, tag="pt")
                            nc.scalar.activation(
                                pt[:, lo:512], sps[:, lo:512], AF.Exp
                            )
                            sgrp.append((st, lo, pt))
                        av_last = None
                        for st, lo, pt in sgrp:
                            av_last = nc.tensor.matmul(
                                yps[:, lo:512],
                                va_sb[st][:, h, :],
                                pt[:, lo:512],
                                start=(st == 0),
                                stop=(st == nst - 1),
                            )
                        if pend and (
                            g == 4
                            or (g == 0 and nst == 4 and len(pend) >= 2)
                        ):
                            _flush_tail(pend.pop(0), av_last)
                    # evacuate the accumulator to SBUF on ACT so the PSUM bank
                    # frees immediately; reciprocal + divide run from SBUF
                    yr = stgp.tile([65, 512], f32, tag="yr", bufs=3, name="yr")
                    nc.scalar.activation(yr[:], yps[:], AF.Copy)
                    rec = recp.tile([1, 512], f16, tag="rec", bufs=3)
                    with nc.allow_low_precision(reason="fp16 softmax recip"):
                        nc.vector.reciprocal(rec[:], yr[64:65, :])
                    pend.append((rec, yr, hp, qrow, tcx))

            # final chunk: remaining tails (short PE stall) + last oproj
            while pend:
                _flush_tail(pend.pop(0), None)
            _oproj(NT - 1)
    return _split_excess_waits(nc, max_waits=1)


def _prep_in_maps(x, W_qkv, b_qkv, W_out):
    f16 = np.float16
    x = np.asarray(x, np.float32)
    W_qkv = np.asarray(W_qkv, np.float32)
    b_qkv = np.asarray(b_qkv, np.float32)
    W_out = np.asarray(W_out, np.float32)

    mask = np.where(
        np.triu(np.ones((128, 128), dtype=bool)), 0.0, -30000.0
    ).astype(f16)
    ident = np.eye(128, dtype=f16)
    ones1 = np.ones((1, 128), dtype=f16)
    in_maps = []
    for c in range(8):
        b, hg = divmod(c, 2)
        qs = slice(hg * 512, (hg + 1) * 512)
        ks = slice(E + hg * 512, E + (hg + 1) * 512)
        vs = slice(2 * E + hg * 512, 2 * E + (hg + 1) * 512)
        wqk_c = np.concatenate(
            [W_qkv[:, qs] * 0.125, W_qkv[:, ks]], axis=1
        ).astype(f16)
        bqk_c = np.concatenate(
            [b_qkv[qs] * 0.125, b_qkv[ks]]
        ).astype(np.float32).reshape(8, 128).T.copy()
        in_maps.append({
            "xt": np.ascontiguousarray(x[b].T).astype(f16),
            "ident": ident,
            "wqk": wqk_c,
            "bqk": bqk_c,
            "wv": W_qkv[:, vs].astype(f16),
            "bv": b_qkv[vs].astype(f16).reshape(1, 512),
            "wo": W_out[hg * 512:(hg + 1) * 512, :].astype(f16),
            "mask": mask,
            "ones1": ones1,
        })
    return in_maps


def run(x, W_qkv, b_qkv, W_out, b_out, trace=False, **trace_kwargs):
    from concourse.bass_utils import run_bass_kernel_spmd

    if "nc" not in _CACHE:
        _CACHE["nc"] = _build()
    nc = _CACHE["nc"]
    in_maps = _prep_in_maps(x, W_qkv, b_qkv, W_out)
    res = run_bass_kernel_spmd(
        nc, in_maps, list(range(8)), trace=trace, **trace_kwargs
    )
    parts = [r["out"] for r in res.results]
    b_out = np.asarray(b_out, np.float32)
    y = np.stack([parts[2 * b] + parts[2 * b + 1] for b in range(B)]) + b_out
    return y.astype(np.float32), res


def kernel(x, W_qkv, b_qkv, W_out, b_out):
    y, _ = run(x, W_qkv, b_qkv, W_out, b_out, trace=False)
    return y
